# revision 5
# baseline (speedup 1.0000x reference)
"""Trainium2 Bass kernel: batched bilinear form  out[n] = elg[n] @ W @ eth[n].

Problem: elg, eth [32768, 1024] fp32, W [1024, 1024] fp32.
Sharding: data-parallel over the batch (N) axis across 8 NeuronCores;
W is replicated.  Per core (4096 rows):

    T      = elg @ W                   (TensorE, fp16 in, fp32 PSUM accum)
    out[n] = sum_e T[n,e] * eth[n,e]   (VectorE fused multiply-reduce, fp32)

elg and W are cast to fp16 on the host (values are ~N(0,1); input
quantization error is ~3e-4 relative per element and ~3e-4 of the output
absmax after accumulation — PSUM accumulation itself is fp32).  eth stays
fp32 and the reduction is fp32.

Layout: the matmul contracts over the partition axis, so elg tiles are
loaded pre-transposed [d, n] straight from HBM via the DMA xbar transpose
(2-byte dtype path) — no PE/DVE transpose work at all.  W lives in SBUF
fp16 [128, 8, 1024] for the whole kernel.  Each 128-row output tile takes
16 matmuls (8 k-tiles x 2 psum half-banks of 512 fp32) followed by one
fused affine_mul_reduce against eth.
"""

import numpy as np

N_TOTAL = 32768
D = 1024
N_CORES = 8
N_CORE = N_TOTAL // N_CORES          # 4096 rows per core
P = 128                              # SBUF/PSUM partitions
K_TILES = D // P                     # 8 contraction tiles
CHUNK_ROWS = 1024                    # rows per DMA chunk
TILES_PER_CHUNK = CHUNK_ROWS // P    # 8
E_HALF = 512                         # fp32 free elems per PSUM bank

_CACHE = {}


def _build_program(n_core_rows):
    import concourse.tile as tile
    from concourse import bacc, mybir

    f16 = mybir.dt.float16
    f32 = mybir.dt.float32

    assert n_core_rows % CHUNK_ROWS == 0
    n_chunks = n_core_rows // CHUNK_ROWS
    n_tiles = n_core_rows // P

    nc = bacc.Bacc("TRN2", target_bir_lowering=False, debug=False)
    elg16 = nc.dram_tensor("elg16", [n_core_rows, D], f16, kind="ExternalInput").ap()
    eth = nc.dram_tensor("eth", [n_core_rows, D], f32, kind="ExternalInput").ap()
    w16 = nc.dram_tensor("w16", [D, D], f16, kind="ExternalInput").ap()
    out = nc.dram_tensor("out", [P, n_tiles], f32, kind="ExternalOutput").ap()

    with tile.TileContext(nc) as tc:
        with tc.tile_pool(name="w_pool", bufs=1) as w_pool, \
             tc.tile_pool(name="lg_pool", bufs=2) as lg_pool, \
             tc.tile_pool(name="et_pool", bufs=2) as et_pool, \
             tc.tile_pool(name="acc_pool", bufs=1) as acc_pool, \
             tc.tile_pool(name="ps_pool", bufs=3, space="PSUM") as ps_pool:

            w_sb = w_pool.tile([P, K_TILES, D], f16, name="w_sb")
            nc.sync.dma_start(out=w_sb[:], in_=w16.rearrange("(k p) e -> p k e", p=P))

            out_sb = acc_pool.tile([P, n_tiles], f32, name="out_sb")

            for c in range(n_chunks):
                r0 = c * CHUNK_ROWS
                elgT = lg_pool.tile([P, K_TILES, CHUNK_ROWS], f16, name="elgT")
                for k in range(K_TILES):
                    nc.sync.dma_start(
                        out=elgT[:, k, :],
                        in_=elg16[r0:r0 + CHUNK_ROWS, k * P:(k + 1) * P],
                        transpose=True,
                    )
                eth_sb = et_pool.tile([P, TILES_PER_CHUNK, D], f32, name="eth_sb")
                nc.sync.dma_start(
                    out=eth_sb[:],
                    in_=eth[r0:r0 + CHUNK_ROWS, :].rearrange("(s p) e -> p s e", p=P),
                )

                for s in range(TILES_PER_CHUNK):
                    t_ps = ps_pool.tile([P, D], f32, name="t_ps")
                    for k in range(K_TILES):
                        for eh in range(2):
                            nc.tensor.matmul(
                                t_ps[:, eh * E_HALF:(eh + 1) * E_HALF],
                                elgT[:, k, s * P:(s + 1) * P],
                                w_sb[:, k, eh * E_HALF:(eh + 1) * E_HALF],
                                start=(k == 0),
                                stop=(k == K_TILES - 1),
                            )
                    t_idx = c * TILES_PER_CHUNK + s
                    prod = lg_pool.tile([P, D], f32, name="prod", bufs=2)
                    nc.vector.affine_mul_reduce(
                        out=prod[:],
                        accum_out=out_sb[:, t_idx:t_idx + 1],
                        in0=t_ps[:],
                        in1=eth_sb[:, s, :],
                        scale=1.0,
                        bias=0.0,
                    )

            nc.sync.dma_start(out=out, in_=out_sb[:])

    nc.compile()
    return nc


def _make_runner(nc, n_cores):
    """Mirror bass2jax.run_bass_via_pjrt's multi-core branch, but return a
    cached jitted callable so repeat calls skip retracing.
    """
    import jax
    import concourse.mybir as mybir
    from concourse import bass2jax
    from jax.experimental.shard_map import shard_map
    from jax.sharding import Mesh, PartitionSpec

    bass2jax.install_neuronx_cc_hook()
    assert nc.dbg_addr is None
    partition_name = nc.partition_id_tensor.name if nc.partition_id_tensor else None

    in_names, out_names, out_avals = [], [], []
    for alloc in nc.m.functions[0].allocations:
        if not isinstance(alloc, mybir.MemoryLocationSet):
            continue
        name = alloc.memorylocations[0].name
        if alloc.kind == "ExternalInput":
            if name != partition_name:
                in_names.append(name)
        elif alloc.kind == "ExternalOutput":
            shape = tuple(alloc.tensor_shape)
            dtype = mybir.dt.np(alloc.dtype)
            out_names.append(name)
            out_avals.append(jax.core.ShapedArray(shape, dtype))
    n_params = len(in_names)
    n_outs = len(out_avals)
    all_in_names = in_names + out_names
    if partition_name is not None:
        all_in_names = all_in_names + [partition_name]

    def _body(*args):
        operands = list(args)
        if partition_name is not None:
            operands.append(bass2jax.partition_id_tensor())
        outs = bass2jax._bass_exec_p.bind(
            *operands,
            out_avals=tuple(out_avals),
            in_names=tuple(all_in_names),
            out_names=tuple(out_names),
            lowering_input_output_aliases=(),
            sim_require_finite=True,
            sim_require_nnan=True,
            nc=nc,
        )
        return tuple(outs)

    devices = jax.devices()[:n_cores]
    assert len(devices) == n_cores
    mesh = Mesh(np.asarray(devices), ("core",))
    spec = PartitionSpec("core")
    sharded = jax.jit(
        shard_map(
            _body,
            mesh=mesh,
            in_specs=(spec,) * (n_params + n_outs),
            out_specs=(spec,) * n_outs,
            check_rep=False,
        ),
        donate_argnums=tuple(range(n_params, n_params + n_outs)),
        keep_unused=True,
    )
    zero_out_shapes = [
        ((n_cores * av.shape[0],) + tuple(av.shape[1:]), av.dtype) for av in out_avals
    ]
    return sharded, in_names, out_names, zero_out_shapes, mesh, spec


def _get_runner():
    r = _CACHE.get("runner")
    if r is None:
        nc = _build_program(N_CORE)
        r = _CACHE["runner"] = _make_runner(nc, N_CORES)
    return r


def _global_inputs(elg, eth, weight):
    """Host-side marshalling: cast + per-core-tile the global arrays."""
    elg16 = elg.astype(np.float16)
    w16 = np.broadcast_to(weight.astype(np.float16), (N_CORES, D, D)).reshape(
        N_CORES * D, D
    )
    return {"elg16": elg16, "eth": eth, "w16": w16}


def _call_runner(global_ins):
    sharded, in_names, out_names, zero_out_shapes, _, _ = _get_runner()
    zeros = [np.zeros(shape, dt) for shape, dt in zero_out_shapes]
    out_arrs = sharded(*[global_ins[n] for n in in_names], *zeros)
    out_g = np.asarray(out_arrs[out_names.index("out")])  # [8*128, 32]
    return np.concatenate(
        [out_g[c * P:(c + 1) * P].T.reshape(-1) for c in range(N_CORES)]
    ).astype(np.float32)


def kernel(elg, eth, weight):
    elg = np.asarray(elg, dtype=np.float32)
    eth = np.asarray(eth, dtype=np.float32)
    weight = np.asarray(weight, dtype=np.float32)
    return _call_runner(_global_inputs(elg, eth, weight))
